# revision 1
# baseline (speedup 1.0000x reference)
"""Trainium2 Bass kernel for nn_Decoder_20486994002617.

8-core tensor-parallel 2-layer llama-style decoder with ragged token-merge
(handled on host), returning the masked-mean cross-entropy loss.

Device layout choices:
  - h (residual) lives in SBUF as [128 part, 8 seq-tiles, 4096] bf16.
  - RMSNorm weights are folded into the consumer weight matrices on host,
    so the device norm is x * rsqrt(mean(x^2)+eps) only; the multiply by
    the per-row factor is fused into the seq->feature transpose as a
    matmul against diag(factor).
  - Attention: heads sharded 4 q-heads + 1 kv-head per core (GQA groups
    align), scores/softmax per (head, 128-row tile), causal mask added via
    an extra accumulating matmul (I.T @ cmask), attn probs transposed back
    through the PE with diag(1/sumexp) fused.
  - MLP: intermediate dim sharded 1376/core, padded to 1408 = 11*128.
  - lm_head: vocab sharded 4000/core; softmax stats (row max, sum-exp) are
    AllReduce'd; the target logit is computed via a host-gathered column
    matrix (wsel) so no device gather is needed.
Outputs per core: gmax [128,8] f32, gsum [128,8] f32, tlog [1,1024] f32.
Host finishes: ce = gmax + log(gsum) - tlog; loss = masked mean.
"""
import numpy as np
import ml_dtypes

from contextlib import ExitStack

import concourse.bass as bass
import concourse.bacc as bacc
import concourse.mybir as mybir
import concourse.tile as tile
from concourse.bass_utils import run_bass_kernel_spmd

F32 = mybir.dt.float32
BF16 = mybir.dt.bfloat16
AF = mybir.ActivationFunctionType
ALU = mybir.AluOpType
AX = mybir.AxisListType

H, HD, NH, NKV = 4096, 128, 32, 8
L, V, S, I = 2, 32000, 1024, 11008
EPS, THETA = 1e-6, 10000.0
NC_ = 8          # cores
IPC = I // NC_   # 1376
IP = 1408        # padded intermediate per core = 11 * 128
VS = V // NC_    # 4000 vocab per core
NEG = -1e9

bf16 = ml_dtypes.bfloat16

last_run_info = {}
_cache = {}


# ----------------------------------------------------------------- device --

def _norm_transpose(nc, pools, h_ap, dst, ident_sb, uid, nt_tag="nt_ps", nt_bufs=2):
    """dst[:, k, :] (32 chunks of [128,128]) = normalized transpose of
    h_ap ([128 seq rows, 4096]). dst free dims must be (32, 128)."""
    small, ntmp, psum = pools
    ssq = small.tile([128, 1], F32, tag="nt_ssq", bufs=2, name=f"ssq_{uid}")
    # Square scratch output goes into dst (overwritten by the transpose after)
    nc.scalar.activation(dst, h_ap.rearrange("p (k m) -> p k m", k=32),
                         AF.Square, accum_out=ssq[:])
    var = small.tile([128, 1], F32, tag="nt_var", bufs=2, name=f"var_{uid}")
    nc.vector.tensor_scalar(var[:], ssq[:], 1.0 / H, EPS, op0=ALU.mult, op1=ALU.add)
    std = small.tile([128, 1], F32, tag="nt_std", bufs=2, name=f"std_{uid}")
    nc.scalar.sqrt(std[:], var[:])
    fac = small.tile([128, 1], F32, tag="nt_fac", bufs=2, name=f"fac_{uid}")
    nc.vector.reciprocal(fac[:], std[:])
    diag = ntmp.tile([128, 128], BF16, tag="nt_diag", bufs=2, name=f"diag_{uid}")
    nc.vector.tensor_scalar_mul(diag[:], ident_sb[:], fac[:])
    for kk in range(8):
        pnt = psum.tile([128, 512], F32, tag=nt_tag, bufs=nt_bufs,
                        name=f"pnt_{uid}_{kk}")
        for j in range(4):
            k = kk * 4 + j
            nc.tensor.matmul(pnt[:, j * 128:(j + 1) * 128],
                             h_ap[:, k * 128:(k + 1) * 128], diag[:],
                             start=True, stop=True)
        nc.any.tensor_copy(dst[:, kk * 4:(kk + 1) * 4, :],
                           pnt[:].rearrange("p (j m) -> p j m", j=4))


def _rope(nc, pools, ps, cos_ap, sf_ap, out, nheads, i):
    """out (bf16 [128, nheads*128]) = rope(ps) with ps a psum slice."""
    small, ntmp, psum = pools
    n = nheads * 128
    t1 = ntmp.tile([128, 512], F32, tag="rope_t1", bufs=1, name=f"t1_{i}_{nheads}")
    t2 = ntmp.tile([128, 512], F32, tag="rope_t2", bufs=1, name=f"t2_{i}_{nheads}")
    nc.vector.tensor_mul(t1[:, :n], ps, cos_ap)
    for hh in range(nheads):
        b = hh * 128
        nc.vector.tensor_mul(t2[:, b:b + 64], ps[:, b + 64:b + 128],
                             sf_ap[:, b:b + 64])
        nc.vector.tensor_mul(t2[:, b + 64:b + 128], ps[:, b:b + 64],
                             sf_ap[:, b + 64:b + 128])
    nc.vector.tensor_add(out[:], t1[:, :n], t2[:, :n])


def build_nc():
    nc = bacc.Bacc("TRN2", target_bir_lowering=False, debug=False,
                   num_devices=NC_)

    din = {}
    def dram_in(name, shape):
        din[name] = nc.dram_tensor(name, shape, BF16, kind="ExternalInput")
        return din[name]

    h0_d = dram_in("h0", [S, H])
    cos4_d = dram_in("cos4", [S, 512])
    sf4_d = dram_in("sf4", [S, 512])
    ident_d = dram_in("ident", [128, 128])
    cmask_d = dram_in("cmask", [128, 128])
    ones_d = dram_in("ones", [128, 1])
    for l in range(L):
        dram_in(f"qw{l}", [H, 512])
        dram_in(f"kvw{l}", [H, 256])
        dram_in(f"ow{l}", [512, H])
        dram_in(f"gw{l}", [H, IP])
        dram_in(f"uw{l}", [H, IP])
        dram_in(f"dw{l}", [IP, H])
    lmw_d = dram_in("lmw", [8, H, VS // 8])
    wsel_d = dram_in("wsel", [H, S])

    gmax_o = nc.dram_tensor("gmax_o", [128, 8], F32, kind="ExternalOutput")
    gsum_o = nc.dram_tensor("gsum_o", [128, 8], F32, kind="ExternalOutput")
    tlog_o = nc.dram_tensor("tlog_o", [1, S], F32, kind="ExternalOutput")

    rg = [list(range(NC_))]

    with tile.TileContext(nc) as tc:
        with (
            tc.tile_pool(name="pconst", bufs=1) as pconst,
            tc.tile_pool(name="psmall", bufs=1) as psmall,
            tc.tile_pool(name="pdram", bufs=1, space="DRAM") as pdram,
        ):
            ident_sb = pconst.tile([128, 128], BF16)
            cmask_sb = pconst.tile([128, 128], BF16)
            ones_sb = pconst.tile([128, 1], BF16)
            cos4_sb = pconst.tile([128, 8, 512], BF16)
            sf4_sb = pconst.tile([128, 8, 512], BF16)
            nc.sync.dma_start(ident_sb[:], ident_d.ap())
            nc.sync.dma_start(cmask_sb[:], cmask_d.ap())
            nc.sync.dma_start(ones_sb[:], ones_d.ap())
            for i in range(8):
                nc.sync.dma_start(cos4_sb[:, i, :], cos4_d.ap()[i * 128:(i + 1) * 128, :])
                nc.sync.dma_start(sf4_sb[:, i, :], sf4_d.ap()[i * 128:(i + 1) * 128, :])

            hstack = ExitStack()
            phh = hstack.enter_context(tc.tile_pool(name="phh", bufs=1))
            if True:
                h_sb = phh.tile([128, 8, H], BF16)
                for i in range(8):
                    nc.sync.dma_start(h_sb[:, i, :], h0_d.ap()[i * 128:(i + 1) * 128, :])

                ar_ins, ar_outss, ar2_ins, ar2_outss = [], [], [], []
                for l in range(L):
                    ar_ins.append(pdram.tile([S, H], BF16, tag=f"ar_in_{l}",
                                             name=f"ar_in_{l}"))
                    ar_outss.append([pdram.tile([512, H], BF16, addr_space="Shared",
                                                tag=f"ar_out_{l}_{c}",
                                                name=f"ar_out_{l}_{c}")
                                     for c in range(2)])
                    ar2_ins.append(pdram.tile([S, H], BF16, tag=f"ar2_in_{l}",
                                              name=f"ar2_in_{l}"))
                    ar2_outss.append([pdram.tile([512, H], BF16, addr_space="Shared",
                                                 tag=f"ar2_out_{l}_{c}",
                                                 name=f"ar2_out_{l}_{c}")
                                      for c in range(2)])

                for l in range(L):
                    # ======== attention: per-tile qkv -> heads -> o-proj ====
                    with (
                        tc.tile_pool(name="pal", bufs=1) as pal,
                        tc.tile_pool(name="paps", bufs=1, space="PSUM") as paps,
                    ):
                        kT_sb = pal.tile([128, S], BF16)
                        v_sb = pal.tile([128, 8, 128], BF16)
                        ar_in = ar_ins[l]
                        ar_outs = ar_outss[l]
                        pools = (psmall, pal, paps)
                        wq_sb = pal.tile([128, 32, 512], BF16)
                        wkv_sb = pal.tile([128, 32, 256], BF16)
                        ow_sb = pal.tile([128, 4, H], BF16)
                        nc.sync.dma_start(
                            wq_sb[:], din[f"qw{l}"].ap().rearrange("(k p) n -> p k n", p=128))
                        nc.sync.dma_start(
                            wkv_sb[:], din[f"kvw{l}"].ap().rearrange("(k p) n -> p k n", p=128))
                        nc.sync.dma_start(
                            ow_sb[:], din[f"ow{l}"].ap().rearrange("(t p) n -> p t n", p=128))
                        for i in range(8):
                            if l > 0:
                                rt = pal.tile([128, H], BF16, tag="resprev",
                                              bufs=1, name=f"resprev_{l}_{i}")
                                nc.sync.dma_start(
                                    rt[:],
                                    ar2_outss[l - 1][i // 4][(i % 4) * 128:(i % 4 + 1) * 128, :])
                                nc.vector.tensor_add(h_sb[:, i, :], h_sb[:, i, :], rt[:])
                            qT_sb = pal.tile([128, 4, 128], BF16, tag="qT",
                                             bufs=2, name=f"qT_{l}_{i}")
                            oT_sb = pal.tile([128, 4, 128], BF16, tag="oT",
                                             bufs=2, name=f"oT_{l}_{i}")
                            xnt = pal.tile([128, 32, 128], BF16, tag="xnt",
                                           bufs=1, name=f"xnt_{l}_{i}")
                            _norm_transpose(nc, pools, h_sb[:, i, :], xnt, ident_sb,
                                            f"a{l}_{i}", nt_bufs=1)
                            psq = paps.tile([128, 512], F32, tag="psq", bufs=1,
                                            name=f"psq_{l}_{i}")
                            pskv = paps.tile([128, 256], F32, tag="pskv", bufs=1,
                                             name=f"pskv_{l}_{i}")
                            for k in range(32):
                                nc.tensor.matmul(psq[:], xnt[:, k, :], wq_sb[:, k, :],
                                                 start=(k == 0), stop=(k == 31))
                                nc.tensor.matmul(pskv[:], xnt[:, k, :], wkv_sb[:, k, :],
                                                 start=(k == 0), stop=(k == 31))
                            q_rot = pal.tile([128, 512], BF16, tag="q_rot", bufs=2,
                                             name=f"qr_{l}_{i}")
                            k_rot = pal.tile([128, 128], BF16, tag="k_rot", bufs=2,
                                             name=f"kr_{l}_{i}")
                            _rope(nc, pools, psq[:], cos4_sb[:, i, :], sf4_sb[:, i, :],
                                  q_rot, 4, f"{l}_{i}")
                            _rope(nc, pools, pskv[:, 0:128], cos4_sb[:, i, 0:128],
                                  sf4_sb[:, i, 0:128], k_rot, 1, f"{l}_{i}")
                            nc.any.tensor_copy(v_sb[:, i, :], pskv[:, 128:256])
                            for hh in range(4):
                                ptr = paps.tile([128, 512], F32, tag="mix", bufs=2,
                                                name=f"ptrq_{l}_{i}_{hh}")
                                nc.tensor.matmul(ptr[:, :128], q_rot[:, hh * 128:(hh + 1) * 128],
                                                 ident_sb[:], start=True, stop=True)
                                nc.any.tensor_copy(qT_sb[:, hh, :], ptr[:, :128])
                            ptrk = paps.tile([128, 512], F32, tag="mix", bufs=2,
                                             name=f"ptrk_{l}_{i}")
                            nc.tensor.matmul(ptrk[:, :128], k_rot[:], ident_sb[:],
                                             start=True, stop=True)
                            nc.any.tensor_copy(kT_sb[:, i * 128:(i + 1) * 128], ptrk[:, :128])
                            n2 = 128 * (i + 1)
                            for hh in range(4):
                                pss = paps.tile([128, 1024], F32, tag="pss", bufs=1,
                                                name=f"pss_{l}_{hh}_{i}")
                                lhs_q = qT_sb[:, hh, :]
                                c0 = 0
                                while c0 < n2 - 128:
                                    N = min(512, n2 - 128 - c0)
                                    nc.tensor.matmul(pss[:, c0:c0 + N], lhs_q,
                                                     kT_sb[:, c0:c0 + N],
                                                     start=True, stop=True)
                                    c0 += N
                                nc.tensor.matmul(pss[:, n2 - 128:n2], lhs_q,
                                                 kT_sb[:, n2 - 128:n2],
                                                 start=True, stop=False)
                                nc.tensor.matmul(pss[:, n2 - 128:n2], ident_sb[:],
                                                 cmask_sb[:], start=False, stop=True)
                                mx = psmall.tile([128, 1], F32, tag="mx", bufs=2,
                                                 name=f"mx_{l}_{hh}_{i}")
                                nc.vector.tensor_reduce(mx[:], pss[:, :n2], axis=AX.X,
                                                        op=ALU.max)
                                negm = psmall.tile([128, 1], F32, tag="negm", bufs=2,
                                                   name=f"negm_{l}_{hh}_{i}")
                                nc.vector.tensor_scalar_mul(negm[:], mx[:], -1.0)
                                sume = psmall.tile([128, 1], F32, tag="sume", bufs=2,
                                                   name=f"sume_{l}_{hh}_{i}")
                                exp_sb = pal.tile([128, 1024], BF16, tag="exp", bufs=1,
                                                  name=f"exp_{l}_{hh}_{i}")
                                nc.scalar.activation(exp_sb[:, :n2], pss[:, :n2], AF.Exp,
                                                     bias=negm[:], accum_out=sume[:])
                                rec = psmall.tile([128, 1], F32, tag="rec", bufs=2,
                                                  name=f"rec_{l}_{hh}_{i}")
                                nc.vector.reciprocal(rec[:], sume[:])
                                diag_r = pal.tile([128, 128], BF16, tag="diag_r", bufs=2,
                                                  name=f"diagr_{l}_{hh}_{i}")
                                nc.vector.tensor_scalar_mul(diag_r[:], ident_sb[:], rec[:])
                                atcol = pal.tile([128, 8, 128], BF16, tag="atcol", bufs=1,
                                                 name=f"atcol_{l}_{hh}_{i}")
                                for j in range(i + 1):
                                    pat = paps.tile([128, 512], F32, tag="mix", bufs=2,
                                                    name=f"pat_{l}_{hh}_{i}_{j}")
                                    nc.tensor.matmul(pat[:, :128], exp_sb[:, j * 128:(j + 1) * 128],
                                                     diag_r[:], start=True, stop=True)
                                    nc.any.tensor_copy(atcol[:, j, :], pat[:, :128])
                                pso = paps.tile([128, 128], F32, tag="pso", bufs=1,
                                                name=f"pso_{l}_{hh}_{i}")
                                for j in range(i + 1):
                                    nc.tensor.matmul(pso[:], v_sb[:, j, :], atcol[:, j, :],
                                                     start=(j == 0), stop=(j == i))
                                nc.any.tensor_copy(oT_sb[:, hh, :], pso[:])
                            ob = pal.tile([128, H], BF16, tag="ob", bufs=1,
                                          name=f"ob_{l}_{i}")
                            for n in range(8):
                                pps = paps.tile([128, 512], F32, tag="mix", bufs=2,
                                                name=f"pop_{l}_{i}_{n}")
                                for t in range(4):
                                    nc.tensor.matmul(pps[:], oT_sb[:, t, :],
                                                     ow_sb[:, t, n * 512:(n + 1) * 512],
                                                     start=(t == 0), stop=(t == 3))
                                nc.any.tensor_copy(ob[:, n * 512:(n + 1) * 512], pps[:])
                            nc.sync.dma_start(ar_in[i * 128:(i + 1) * 128, :], ob[:])
                            if i == 3:
                                nc.gpsimd.collective_compute(
                                    "AllReduce", ALU.add, replica_groups=rg,
                                    ins=[ar_in[0:512, :].opt()], outs=[ar_outs[0].opt()])
                        nc.gpsimd.collective_compute(
                            "AllReduce", ALU.add, replica_groups=rg,
                            ins=[ar_in[512:1024, :].opt()], outs=[ar_outs[1].opt()])

                    # ===== MLP: per-half gate/up -> down -> AR2 =============
                    with (
                        tc.tile_pool(name="pml", bufs=1) as pml,
                        tc.tile_pool(name="pmps", bufs=1, space="PSUM") as pmps,
                    ):
                        ar2_in = ar2_ins[l]
                        ar2_outs = ar2_outss[l]
                        for ig in range(2):
                            with tc.tile_pool(name="pgu", bufs=1) as pgu:
                                pools = (psmall, pgu, pmps)
                                yt_sb = pml.tile([128, 11, 512], BF16, tag="yt",
                                                 bufs=2, name=f"yt_{l}_{ig}")
                                xnts = []
                                for ii in range(4):
                                    i = ig * 4 + ii
                                    rt = pgu.tile([128, H], BF16, tag="resat",
                                                  bufs=1, name=f"resat_{l}_{i}")
                                    nc.sync.dma_start(
                                        rt[:], ar_outs[ig][ii * 128:(ii + 1) * 128, :])
                                    nc.vector.tensor_add(h_sb[:, i, :], h_sb[:, i, :], rt[:])
                                    xnt = pgu.tile([128, 32, 128], BF16, tag="xnt2",
                                                   bufs=4, name=f"xnt2_{l}_{i}")
                                    _norm_transpose(nc, pools, h_sb[:, i, :], xnt, ident_sb,
                                                    f"m{l}_{i}", nt_tag="mlpps", nt_bufs=4)
                                    xnts.append(xnt)
                                gu = {}
                                for wname, tag in ((f"gw{l}", "g"), (f"uw{l}", "u")):
                                    outs = [pgu.tile([128, IP], BF16, tag=tag, bufs=4,
                                                     name=f"{tag}_{l}_{ig}_{ii}")
                                            for ii in range(4)]
                                    gu[tag] = outs
                                    for nb in range(3):
                                        NB = 512 if nb < 2 else IP - 1024
                                        pg = [pmps.tile([128, 512], F32, tag="mlpps", bufs=4,
                                                        name=f"pg_{l}_{ig}_{tag}_{nb}_{ii}")
                                              for ii in range(4)]
                                        for kp in range(8):
                                            wt = pgu.tile([128, 4, 512], BF16, tag="wstream",
                                                          bufs=2,
                                                          name=f"wt_{l}_{ig}_{tag}_{nb}_{kp}")
                                            nc.sync.dma_start(
                                                wt[:, :, :NB],
                                                din[wname].ap()[kp * 512:(kp + 1) * 512,
                                                                nb * 512:nb * 512 + NB]
                                                .rearrange("(j p) n -> p j n", p=128))
                                            for jk in range(4):
                                                k = kp * 4 + jk
                                                for ii in range(4):
                                                    nc.tensor.matmul(pg[ii][:, :NB],
                                                                     xnts[ii][:, k, :],
                                                                     wt[:, jk, :NB],
                                                                     start=(k == 0), stop=(k == 31))
                                        for ii in range(4):
                                            nc.any.tensor_copy(
                                                outs[ii][:, nb * 512:nb * 512 + NB],
                                                pg[ii][:, :NB])
                                for ii in range(4):
                                    i = ig * 4 + ii
                                    ysil = pgu.tile([128, IP], BF16, tag="ysil", bufs=2,
                                                    name=f"ysil_{l}_{i}")
                                    nc.scalar.activation(ysil[:], gu["g"][ii][:], AF.Silu)
                                    y = gu["u"][ii]
                                    nc.vector.tensor_mul(y[:], ysil[:], y[:])
                                    for tq in range(3):
                                        ts = [tq * 4 + j for j in range(4) if tq * 4 + j < 11]
                                        ptr = pmps.tile([128, 512], F32, tag="mlpps", bufs=4,
                                                        name=f"ytr_{l}_{i}_{tq}")
                                        for jj, t in enumerate(ts):
                                            nc.tensor.matmul(ptr[:, jj * 128:(jj + 1) * 128],
                                                             y[:, t * 128:(t + 1) * 128],
                                                             ident_sb[:], start=True, stop=True)
                                        nc.any.tensor_copy(
                                            yt_sb[:, ts[0]:ts[0] + len(ts),
                                                  ii * 128:(ii + 1) * 128],
                                            ptr[:, :len(ts) * 128].rearrange(
                                                "p (j m) -> p j m", j=len(ts)))
                                for n in range(8):
                                    pd = [pmps.tile([128, 512], F32, tag=f"pd{ii}", bufs=1,
                                                    name=f"pd_{l}_{ig}_{n}_{ii}")
                                          for ii in range(4)]
                                    for tp in range(3):
                                        nt = 4 if tp < 2 else 3
                                        dwt = pgu.tile([128, 4, 512], BF16, tag="dwstream",
                                                       bufs=2, name=f"dwt_{l}_{ig}_{n}_{tp}")
                                        nc.sync.dma_start(
                                            dwt[:, :nt, :],
                                            din[f"dw{l}"].ap()[tp * 512:tp * 512 + nt * 128,
                                                               n * 512:(n + 1) * 512]
                                            .rearrange("(j p) n -> p j n", p=128))
                                        for jt in range(nt):
                                            t = tp * 4 + jt
                                            for ii in range(4):
                                                nc.tensor.matmul(
                                                    pd[ii][:], yt_sb[:, t, ii * 128:(ii + 1) * 128],
                                                    dwt[:, jt, :], start=(t == 0), stop=(t == 10))
                                    for ii in range(4):
                                        i = ig * 4 + ii
                                        db = pgu.tile([128, 512], BF16, tag="db", bufs=4,
                                                      name=f"db_{l}_{ig}_{n}_{ii}")
                                        nc.any.tensor_copy(db[:], pd[ii][:])
                                        nc.sync.dma_start(
                                            ar2_in[i * 128:(i + 1) * 128,
                                                   n * 512:(n + 1) * 512], db[:])
                                nc.gpsimd.collective_compute(
                                    "AllReduce", ALU.add, replica_groups=rg,
                                    ins=[ar2_in[ig * 512:(ig + 1) * 512, :].opt()],
                                    outs=[ar2_outs[ig].opt()])

                # spill h to DRAM so the h pool can close before lm phase
                hdram = pdram.tile([S, H], BF16)
                for i in range(8):
                    nc.gpsimd.dma_start(hdram[i * 128:(i + 1) * 128, :], h_sb[:, i, :])
            hstack.close()  # release h pool

            # ======================= final norm -> xf ======================
            with tc.tile_pool(name="pxf", bufs=1) as pxf:
                xf_sb = pxf.tile([128, 32, S], BF16)
                with (
                    tc.tile_pool(name="pfn", bufs=1) as pfn,
                    tc.tile_pool(name="pfps", bufs=1, space="PSUM") as pfps,
                ):
                    pools = (psmall, pfn, pfps)
                    for i in range(8):
                        ht = pfn.tile([128, H], BF16, tag="hfin", bufs=2,
                                      name=f"hfin_{i}")
                        nc.gpsimd.dma_start(ht[:], hdram[i * 128:(i + 1) * 128, :])
                        rt = pfn.tile([128, H], BF16, tag="resfin", bufs=2,
                                      name=f"resfin_{i}")
                        nc.sync.dma_start(
                            rt[:], ar2_outss[L - 1][i // 4][(i % 4) * 128:(i % 4 + 1) * 128, :])
                        nc.vector.tensor_add(ht[:], ht[:], rt[:])
                        dst = xf_sb[:, :, i * 128:(i + 1) * 128]
                        _norm_transpose(nc, pools, ht[:], dst, ident_sb, f"f{i}")
                self_lm_phases(nc, tc, psmall, xf_sb, ident_sb, ones_sb,
                               wsel_d, lmw_d, tlog_o, gmax_o, gsum_o, rg)

    nc.compile()
    return nc


def self_lm_phases(nc, tc, psmall, xf_sb, ident_sb, ones_sb, wsel_d, lmw_d,
                   tlog_o, gmax_o, gsum_o, rg):
            if True:
                pass
            with (
                tc.tile_pool(name="ptl", bufs=1) as ptl,
                tc.tile_pool(name="ptps", bufs=1, space="PSUM") as ptps,
            ):
                pt0 = ptps.tile([1, 512], F32)
                pt1 = ptps.tile([1, 512], F32)
                for kp in range(8):
                    ws = ptl.tile([128, 4, S], BF16, tag="wsel", bufs=2, name=f"ws_{kp}")
                    nc.sync.dma_start(
                        ws[:], wsel_d.ap()[kp * 512:(kp + 1) * 512, :]
                        .rearrange("(j p) n -> p j n", p=128))
                    for jk in range(4):
                        k = kp * 4 + jk
                        tm = ptl.tile([128, S], BF16, tag="tm", bufs=2, name=f"tm_{k}")
                        nc.vector.tensor_mul(tm[:], xf_sb[:, k, :], ws[:, jk, :])
                        nc.tensor.matmul(pt0[:], ones_sb[:], tm[:, :512],
                                         start=(k == 0), stop=(k == 31))
                        nc.tensor.matmul(pt1[:], ones_sb[:], tm[:, 512:],
                                         start=(k == 0), stop=(k == 31))
                tl_sb = ptl.tile([1, S], F32)
                nc.any.tensor_copy(tl_sb[:, :512], pt0[:])
                nc.any.tensor_copy(tl_sb[:, 512:], pt1[:])
                nc.sync.dma_start(tlog_o.ap(), tl_sb[:])

            with (
                tc.tile_pool(name="plm", bufs=1) as plm,
                tc.tile_pool(name="plps", bufs=1, space="PSUM") as plps,
                tc.tile_pool(name="pld", bufs=1, space="DRAM") as pld,
            ):
                logits = [plm.tile([128, VS], BF16, tag=f"lg{i}", bufs=1,
                                   name=f"logits_{i}") for i in range(8)]
                for vb in range(8):
                    pl = [plps.tile([128, 500], F32, tag=f"pl{i}", bufs=1,
                                    name=f"pl_{vb}_{i}") for i in range(8)]
                    for kp in range(8):
                        lt = plm.tile([128, 4, 500], BF16, tag="lmw", bufs=4,
                                      name=f"lt_{vb}_{kp}")
                        nc.sync.dma_start(
                            lt[:], lmw_d.ap()[vb, kp * 512:(kp + 1) * 512, :]
                            .rearrange("(j p) n -> p j n", p=128))
                        for jk in range(4):
                            k = kp * 4 + jk
                            for i in range(8):
                                nc.tensor.matmul(pl[i][:], xf_sb[:, k, i * 128:(i + 1) * 128],
                                                 lt[:, jk, :], start=(k == 0), stop=(k == 31))
                    for i in range(8):
                        nc.any.tensor_copy(logits[i][:, vb * 500:(vb + 1) * 500], pl[i][:])

                gmax_sb = plm.tile([128, 8], F32)
                for i in range(8):
                    nc.vector.tensor_reduce(gmax_sb[:, i:i + 1], logits[i][:],
                                            axis=AX.X, op=ALU.max)
                gm_in = pld.tile([128, 8], F32)
                gm_out = pld.tile([128, 8], F32, addr_space="Shared")
                nc.sync.dma_start(gm_in[:], gmax_sb[:])
                nc.gpsimd.collective_compute("AllReduce", ALU.max, replica_groups=rg,
                                             ins=[gm_in.opt()], outs=[gm_out.opt()])
                gm_sb = plm.tile([128, 8], F32)
                nc.sync.dma_start(gm_sb[:], gm_out[:])
                nc.sync.dma_start(gmax_o.ap(), gm_sb[:])
                negg = plm.tile([128, 8], F32)
                nc.vector.tensor_scalar_mul(negg[:], gm_sb[:], -1.0)
                gs_sb = plm.tile([128, 8], F32)
                for i in range(8):
                    scr = plm.tile([128, VS], BF16, tag="scr", bufs=2, name=f"scr_{i}")
                    nc.scalar.activation(scr[:], logits[i][:], AF.Exp,
                                         bias=negg[:, i:i + 1],
                                         accum_out=gs_sb[:, i:i + 1])
                gs_in = pld.tile([128, 8], F32)
                gs_out = pld.tile([128, 8], F32, addr_space="Shared")
                nc.sync.dma_start(gs_in[:], gs_sb[:])
                nc.gpsimd.collective_compute("AllReduce", ALU.add, replica_groups=rg,
                                             ins=[gs_in.opt()], outs=[gs_out.opt()])
                gsf_sb = plm.tile([128, 8], F32)
                nc.sync.dma_start(gsf_sb[:], gs_out[:])
                nc.sync.dma_start(gsum_o.ap(), gsf_sb[:])


# ------------------------------------------------------------------- host --

def host_prep(inputs):
    inp = {k: np.asarray(v) for k, v in inputs.items()}
    embed = inp["embed"].astype(np.float32)
    ids = inp["input_ids"].reshape(-1).astype(np.int64)
    labels = inp["labels"].reshape(-1).astype(np.int64)

    h = embed[ids]
    cw = inp["conv_w"].astype(np.float32)
    logit = h[:-1] @ cw[0, :H] + h[1:] @ cw[0, H:] + np.float32(inp["conv_b"][0])
    mask = logit > 0
    m = np.concatenate([mask, [False]])
    hn = np.where(m[:, None], 0.5 * (h + np.roll(h, -1, axis=0)), h)
    keep = np.concatenate([[True], ~mask])
    order = np.argsort(~keep, kind="stable")
    h0 = hn[order]
    lab = labels[order]
    valid_len = int(keep.sum())

    inv = 1.0 / (THETA ** (np.arange(0, HD, 2, dtype=np.float32) / HD))
    t = np.arange(S, dtype=np.float32)
    freqs = np.outer(t, inv)
    emb = np.concatenate([freqs, freqs], -1)
    cos, sin = np.cos(emb), np.sin(emb)
    sinflip = np.concatenate([-sin[:, :HD // 2], sin[:, HD // 2:]], -1)
    cos4 = np.tile(cos, (1, 4)).astype(bf16)
    sinflip4 = np.tile(sinflip, (1, 4)).astype(bf16)

    ident = np.eye(128, dtype=bf16)
    cmask = np.where(np.arange(128)[None, :] > np.arange(128)[:, None],
                     np.float32(NEG), np.float32(0)).astype(bf16)
    ones = np.ones((128, 1), dtype=bf16)

    ln1 = inp["ln1_w"].astype(np.float32)
    ln2 = inp["ln2_w"].astype(np.float32)
    normw = inp["norm_w"].astype(np.float32)
    qsc = np.float32(1.0 / np.sqrt(HD))
    lm_folded = normw[:, None] * inp["lm_head_w"].astype(np.float32)
    tgt = np.concatenate([lab[1:], [0]]).astype(np.int64)
    wsel = np.ascontiguousarray(lm_folded[:, tgt]).astype(bf16)

    common = dict(h0=h0.astype(bf16), cos4=cos4, sf4=sinflip4, ident=ident,
                  cmask=cmask, ones=ones, wsel=wsel)
    in_maps = []
    for c in range(NC_):
        mcore = dict(common)
        for l in range(L):
            qw = ln1[l][:, None] * inp["q_w"][l].astype(np.float32) * qsc
            kw = ln1[l][:, None] * inp["k_w"][l].astype(np.float32)
            vw = ln1[l][:, None] * inp["v_w"][l].astype(np.float32)
            gw = ln2[l][:, None] * inp["gate_w"][l].astype(np.float32)
            uw = ln2[l][:, None] * inp["up_w"][l].astype(np.float32)
            dw = inp["down_w"][l].astype(np.float32)
            gws = np.zeros((H, IP), np.float32)
            uws = np.zeros((H, IP), np.float32)
            dws = np.zeros((IP, H), np.float32)
            gws[:, :IPC] = gw[:, c * IPC:(c + 1) * IPC]
            uws[:, :IPC] = uw[:, c * IPC:(c + 1) * IPC]
            dws[:IPC] = dw[c * IPC:(c + 1) * IPC]
            mcore[f"qw{l}"] = np.ascontiguousarray(qw[:, c * 512:(c + 1) * 512]).astype(bf16)
            mcore[f"kvw{l}"] = np.concatenate(
                [kw[:, c * 128:(c + 1) * 128], vw[:, c * 128:(c + 1) * 128]],
                1).astype(bf16)
            mcore[f"ow{l}"] = np.ascontiguousarray(
                inp["o_w"][l][c * 512:(c + 1) * 512].astype(np.float32)).astype(bf16)
            mcore[f"gw{l}"] = gws.astype(bf16)
            mcore[f"uw{l}"] = uws.astype(bf16)
            mcore[f"dw{l}"] = dws.astype(bf16)
        lmc = lm_folded[:, c * VS:(c + 1) * VS]
        mcore["lmw"] = np.ascontiguousarray(
            lmc.reshape(H, 8, VS // 8).transpose(1, 0, 2)).astype(bf16)
        in_maps.append(mcore)

    return in_maps, valid_len


def kernel(**inputs) -> np.ndarray:
    in_maps, valid_len = host_prep(inputs)
    if "nc" not in _cache:
        _cache["nc"] = build_nc()
    nc = _cache["nc"]
    res = run_bass_kernel_spmd(nc, in_maps, list(range(NC_)),
                               **last_run_info.get("run_kwargs", {}))
    last_run_info["res"] = res
    out = res.results[0]
    gmax = out["gmax_o"].transpose(1, 0).reshape(S).astype(np.float64)
    gsum = out["gsum_o"].transpose(1, 0).reshape(S).astype(np.float64)
    tlog = out["tlog_o"].reshape(S).astype(np.float64)
    ce = gmax + np.log(gsum) - tlog
    w = (np.arange(S - 1) < valid_len - 1).astype(np.float64)
    loss = (ce[:S - 1] * w).sum() / w.sum()
    return np.float32(loss)



# revision 6
# speedup vs baseline: 1.6447x; 1.6447x over previous
"""Trainium2 Bass kernel for nn_Decoder_20486994002617 (fp8 rewrite).

8-core tensor-parallel 2-layer llama-style decoder with ragged token-merge
(host), masked-mean CE loss. All large GEMMs run fp8e4 with DoubleRow
(2 contraction planes/pass). Residual h kept in SBUF as h*8 bf16 so fp8
AllReduce deltas (also *8) add with a single tensor_add.

Scales: h*8 (SH), xnt*16 (SX), qT*128 (SQ), kT*16 (SK), vT*16 (SV),
probs*128 (SP), oT*16 (SO), y*8 (SY), xf*16 (SXF); per-weight pow2 scales
computed on host, descale folded into rope tables / psum-copy scales.

Layout summary (per core):
  h_sb   [128, 8, 4096]  bf16  seq-major residual (x8)
  xnt_sb [128, 32, 1024] fp8   normalized-transposed x (x16), reused as xf
  qkv    weight-stationary: out qT/kT/vT [hd, seq] directly
  MLP    gate/up weight-stationary -> yT [I, seq] fp8; down uses yT as lhsT
  lm     xf-pair stationary, lmw moving, per-vb local-max exp epilogue
Collectives: 4x2 fp8 AllReduce [512,4096] chunks + 2 tiny f32 ARs.
"""
import numpy as np
import ml_dtypes

import concourse.bass as bass
import concourse.bacc as bacc
import concourse.mybir as mybir
import concourse.tile as tile
from concourse.bass_utils import run_bass_kernel_spmd

F32 = mybir.dt.float32
BF16 = mybir.dt.bfloat16
F8 = mybir.dt.float8e4
AF = mybir.ActivationFunctionType
ALU = mybir.AluOpType
AX = mybir.AxisListType
DR = mybir.MatmulPerfMode.DoubleRow

H, HD, NH, NKV = 4096, 128, 32, 8
L, V, S, I = 2, 32000, 1024, 11008
EPS, THETA = 1e-6, 10000.0
NC_ = 8
IPC = I // NC_   # 1376
IP = 1408        # padded per-core intermediate = 11*128
NB = IP // 128   # 11
VS = V // NC_    # 4000
NEG = -1e9

SH, SX, SQ, SK, SV, SP, SO, SY, SXF = 8.0, 16.0, 128.0, 16.0, 16.0, 128.0, 16.0, 8.0, 16.0

bf16 = ml_dtypes.bfloat16
f8 = ml_dtypes.float8_e4m3

last_run_info = {}
_cache = {}


def _pow2(x, target=120.0):
    m = float(np.abs(x).max())
    if m == 0:
        return 1.0
    return float(2.0 ** np.floor(np.log2(target / m)))


# ----------------------------------------------------------------- device --

def _norm_transpose(nc, small, psum, h_ap, xnt_dst, identd_sb, uid, out_scale):
    """xnt_dst (fp8 [128, 32, 128] slice of xnt_sb) = out_scale * normalize(
    h_ap [128 seq, 4096] at x SH) transposed. identd = I/SH."""
    ssq = small.tile([128, 1], F32, tag="nt_ssq", bufs=2, name=f"ssq_{uid}")
    sq_scr = small.tile([128, 4096], BF16, tag="nt_scr", bufs=1, name=f"scr_{uid}")
    nc.scalar.activation(sq_scr[:], h_ap, AF.Square, accum_out=ssq[:])
    var = small.tile([128, 1], F32, tag="nt_var", bufs=2, name=f"var_{uid}")
    nc.vector.tensor_scalar(var[:], ssq[:], 1.0 / (SH * SH * H), EPS,
                            op0=ALU.mult, op1=ALU.add)
    std = small.tile([128, 1], F32, tag="nt_std", bufs=2, name=f"std_{uid}")
    nc.scalar.sqrt(std[:], var[:])
    fac = small.tile([128, 1], F32, tag="nt_fac", bufs=2, name=f"fac_{uid}")
    nc.vector.reciprocal(fac[:], std[:])
    diag = small.tile([128, 128], BF16, tag="nt_diag", bufs=2, name=f"diag_{uid}")
    nc.vector.tensor_scalar_mul(diag[:], identd_sb[:], fac[:])
    for kk in range(8):
        pnt = psum.tile([128, 512], F32, tag="mix", bufs=2, name=f"pnt_{uid}_{kk}")
        for j in range(4):
            k = kk * 4 + j
            nc.tensor.matmul(pnt[:, j * 128:(j + 1) * 128],
                             h_ap[:, k * 128:(k + 1) * 128], diag[:],
                             start=True, stop=True)
        nc.scalar.activation(
            xnt_dst[:, kk * 4:(kk + 1) * 4, :],
            pnt[:].rearrange("p (j m) -> p j m", j=4), AF.Copy, scale=out_scale)


def build_nc(wscales):
    (sqw, skw, svw, sow, sgw, suw, sdw, slm) = wscales
    nc = bacc.Bacc("TRN2", target_bir_lowering=False, debug=False,
                   num_devices=NC_)

    din = {}
    def dram_in(name, shape, dt):
        din[name] = nc.dram_tensor(name, shape, dt, kind="ExternalInput")
        return din[name]

    h0_d = dram_in("h0", [S, H], BF16)
    cosq_d = dram_in("cosq", [128, S], BF16)
    sinq_d = dram_in("sinq", [128, S], BF16)
    cosk_d = dram_in("cosk", [128, S], BF16)
    sink_d = dram_in("sink", [128, S], BF16)
    ident1_d = dram_in("ident1", [128, 128], BF16)
    identd_d = dram_in("identd", [128, 128], BF16)
    ident128_d = dram_in("ident128", [128, 128], BF16)
    cmask_d = dram_in("cmask", [128, 128], BF16)
    ones_d = dram_in("ones", [128, 1], BF16)
    for l in range(L):
        dram_in(f"wqkv{l}", [128, 32, 6, 128], F8)
        dram_in(f"ow{l}", [128, 4, H], F8)
        dram_in(f"gw{l}", [128, NB, 32, 128], F8)
        dram_in(f"uw{l}", [128, NB, 32, 128], F8)
        dram_in(f"dw{l}", [128, NB, H], F8)
    lmw_d = dram_in("lmw", [128, 32, VS], F8)
    wsel_d = dram_in("wsel", [128, 32, S], BF16)

    gmax_o = nc.dram_tensor("gmax_o", [128, 8], F32, kind="ExternalOutput")
    gsum_o = nc.dram_tensor("gsum_o", [128, 8], F32, kind="ExternalOutput")
    tlog_o = nc.dram_tensor("tlog_o", [1, S], F32, kind="ExternalOutput")

    rg = [list(range(NC_))]

    with tile.TileContext(nc) as tc:
        with (
            tc.tile_pool(name="pconst", bufs=1) as pconst,
            tc.tile_pool(name="psmall", bufs=1) as psmall,
            tc.tile_pool(name="pmain", bufs=1) as pmain,
            tc.tile_pool(name="pdram", bufs=1, space="DRAM") as pdram,
        ):
            ident1 = pconst.tile([128, 128], BF16)
            identd = pconst.tile([128, 128], BF16)
            ident128 = pconst.tile([128, 128], BF16)
            cmask = pconst.tile([128, 128], BF16)
            ones_sb = pconst.tile([128, 1], BF16)
            cosq = pconst.tile([128, S], BF16)
            sinq = pconst.tile([128, S], BF16)
            cosk = pconst.tile([128, S], BF16)
            sink = pconst.tile([128, S], BF16)
            nc.sync.dma_start(ident1[:], ident1_d.ap())
            nc.sync.dma_start(identd[:], identd_d.ap())
            nc.sync.dma_start(ident128[:], ident128_d.ap())
            nc.sync.dma_start(cmask[:], cmask_d.ap())
            nc.sync.dma_start(ones_sb[:], ones_d.ap())
            nc.sync.dma_start(cosq[:], cosq_d.ap())
            nc.sync.dma_start(sinq[:], sinq_d.ap())
            nc.sync.dma_start(cosk[:], cosk_d.ap())
            nc.sync.dma_start(sink[:], sink_d.ap())

            h_sb = pmain.tile([128, 8, H], BF16)
            xnt_sb = pmain.tile([128, 32, S], F8)
            for i in range(8):
                nc.sync.dma_start(h_sb[:, i, :], h0_d.ap()[i * 128:(i + 1) * 128, :])

            # collective buffers
            ar_ins, ar_outs = [], []
            for l in range(L):
                row = []
                for ph in ("a", "m"):
                    ain = pdram.tile([S, H], F8, tag=f"ar{ph}_in_{l}",
                                     name=f"ar{ph}_in_{l}")
                    aouts = [pdram.tile([512, H], F8, addr_space="Shared",
                                        tag=f"ar{ph}_out_{l}_{c}",
                                        name=f"ar{ph}_out_{l}_{c}")
                             for c in range(2)]
                    row.append((ain, aouts))
                ar_ins.append(row)

            for l in range(L):
                with (
                    tc.tile_pool(name=f"pa{l}", bufs=1) as pa,
                    tc.tile_pool(name=f"paps{l}", bufs=1, space="PSUM") as paps,
                ):
                    _attention(nc, tc, l, pa, paps, psmall, din, h_sb, xnt_sb,
                               ident1, identd, ident128, cmask,
                               cosq, sinq, cosk, sink,
                               ar_ins[l][0],
                               None if l == 0 else ar_ins[l - 1][1],
                               sqw, skw, svw, sow)
                with (
                    tc.tile_pool(name=f"pm{l}", bufs=1) as pm,
                    tc.tile_pool(name=f"pmps{l}", bufs=1, space="PSUM") as pmps,
                ):
                    _mlp(nc, tc, l, pm, pmps, psmall, din, h_sb, xnt_sb,
                         identd, ar_ins[l][0], ar_ins[l][1], sgw, suw, sdw)

            with (
                tc.tile_pool(name="plm", bufs=1) as plm,
                tc.tile_pool(name="plps", bufs=1, space="PSUM") as plps,
                tc.tile_pool(name="pld", bufs=1, space="DRAM") as pld,
            ):
                _final(nc, tc, plm, plps, pld, psmall, din, h_sb, xnt_sb,
                       identd, ones_sb, ar_ins[L - 1][1],
                       lmw_d, wsel_d, tlog_o, gmax_o, gsum_o, rg, slm)

    nc.compile()
    return nc


def _attention(nc, tc, l, pa, paps, psmall, din, h_sb, xnt_sb,
               ident1, identd, ident128, cmask, cosq, sinq, cosk, sink,
               ar_a, ar_m_prev, sqw, skw, svw, sow):
    rg = [list(range(NC_))]
    ar_in, ar_out = ar_a

    wqkv = pa.tile([128, 32, 6, 128], F8)
    ow_sb = pa.tile([128, 4, H], F8)
    nc.sync.dma_start(wqkv[:], din[f"wqkv{l}"].ap())
    nc.sync.dma_start(ow_sb[:], din[f"ow{l}"].ap())

    qT = pa.tile([128, 4, S], F8)
    kT = pa.tile([128, S], F8)
    vT = pa.tile([128, S], F8)
    v_sb = pa.tile([128, 8, 128], F8)

    # residual + norm + transpose, per seq-half so AR chunks pipeline
    def norm_tile(i):
        if ar_m_prev is not None:
            pin, pouts = ar_m_prev
            rt = pa.tile([128, H], F8, tag="resprev", bufs=1,
                         name=f"resprev_{l}_{i}")
            nc.sync.dma_start(rt[:],
                              pouts[i // 4][(i % 4) * 128:(i % 4 + 1) * 128, :])
            nc.vector.tensor_add(h_sb[:, i, :], h_sb[:, i, :], rt[:])
        _norm_transpose(nc, psmall, paps, h_sb[:, i, :],
                        xnt_sb[:, :, i * 128:(i + 1) * 128],
                        identd, f"a{l}_{i}", SX)

    def qkv_half(half):
        s0 = half * 512
        for b in range(6):
            pq = paps.tile([128, 512], F32, tag="qkv", bufs=2,
                           name=f"pq_{l}_{b}_{half}")
            for kp in range(16):
                nc.tensor.matmul(pq[:], wqkv[:, 2 * kp:2 * kp + 2, b, :],
                                 xnt_sb[:, 2 * kp:2 * kp + 2, s0:s0 + 512],
                                 start=(kp == 0), stop=(kp == 15), perf_mode=DR)
            if b < 4:      # q head -> rope -> qT
                _rope_t(nc, pa, pq, cosq, sinq, qT[:, b, s0:s0 + 512], s0,
                        f"q{l}_{b}_{half}")
            elif b == 4:   # k -> rope -> kT
                _rope_t(nc, pa, pq, cosk, sink, kT[:, s0:s0 + 512], s0,
                        f"k{l}_{half}")
            else:          # v -> vT fp8
                nc.scalar.activation(vT[:, s0:s0 + 512], pq[:], AF.Copy,
                                     scale=SV / (SX * svw))

    def v_detranspose(half):
        for t in range(4 * half, 4 * half + 4):
            pv = paps.tile([128, 512], F32, tag="mix", bufs=2,
                           name=f"pv_{l}_{t}")
            nc.tensor.matmul(pv[:, :128], vT[:, t * 128:(t + 1) * 128],
                             ident1[:], start=True, stop=True)
            nc.any.tensor_copy(v_sb[:, t, :], pv[:, :128])

    def attn_tile(i):
        n2 = 128 * (i + 1)
        oT = pa.tile([128, 4, 128], F8, tag="oT", bufs=2, name=f"oT_{l}_{i}")
        for hh in range(4):
            pss = paps.tile([128, 1024], F32, tag="pss", bufs=2,
                            name=f"pss_{l}_{i}_{hh}")
            lhs_q = qT[:, hh, i * 128:(i + 1) * 128]
            c0 = 0
            while c0 < n2 - 128:
                n = min(512, n2 - 128 - c0)
                nc.tensor.matmul(pss[:, c0:c0 + n], lhs_q, kT[:, c0:c0 + n],
                                 start=True, stop=True)
                c0 += n
            nc.tensor.matmul(pss[:, n2 - 128:n2], lhs_q, kT[:, n2 - 128:n2],
                             start=True, stop=False)
            nc.tensor.matmul(pss[:, n2 - 128:n2], ident1[:], cmask[:],
                             start=False, stop=True)
            mx = psmall.tile([128, 1], F32, tag="mx", bufs=2,
                             name=f"mx_{l}_{i}_{hh}")
            nc.vector.tensor_reduce(mx[:], pss[:, :n2], axis=AX.X, op=ALU.max)
            negm = psmall.tile([128, 1], F32, tag="negm", bufs=2,
                               name=f"negm_{l}_{i}_{hh}")
            nc.vector.tensor_scalar_mul(negm[:], mx[:], -1.0 / (SQ * SK))
            sume = psmall.tile([128, 1], F32, tag="sume", bufs=2,
                               name=f"sume_{l}_{i}_{hh}")
            exp_sb = pa.tile([128, 1024], F8, tag="exp", bufs=2,
                             name=f"exp_{l}_{i}_{hh}")
            nc.scalar.activation(exp_sb[:, :n2], pss[:, :n2], AF.Exp,
                                 bias=negm[:], scale=1.0 / (SQ * SK),
                                 accum_out=sume[:])
            rec = psmall.tile([128, 1], F32, tag="rec", bufs=2,
                              name=f"rec_{l}_{i}_{hh}")
            nc.vector.reciprocal(rec[:], sume[:])
            diag_r = pa.tile([128, 128], F8, tag="diag_r", bufs=2,
                             name=f"diagr_{l}_{i}_{hh}")
            nc.vector.tensor_scalar_mul(diag_r[:], ident128[:], rec[:])
            atcol = pa.tile([128, 8, 128], F8, tag="atcol", bufs=2,
                            name=f"atcol_{l}_{i}_{hh}")
            for j in range(i + 1):
                pat = paps.tile([128, 512], F32, tag="mix", bufs=2,
                                name=f"pat_{l}_{i}_{hh}_{j}")
                nc.tensor.matmul(pat[:, :128], exp_sb[:, j * 128:(j + 1) * 128],
                                 diag_r[:], start=True, stop=True)
                nc.any.tensor_copy(atcol[:, j, :], pat[:, :128])
            pso = paps.tile([128, 512], F32, tag="mix", bufs=2,
                            name=f"pso_{l}_{i}_{hh}")
            npair = (i + 1) // 2
            for jp in range(npair):
                nc.tensor.matmul(pso[:, :128], v_sb[:, 2 * jp:2 * jp + 2, :],
                                 atcol[:, 2 * jp:2 * jp + 2, :],
                                 start=(jp == 0), stop=(jp == npair - 1 and (i + 1) % 2 == 0),
                                 perf_mode=DR)
            if (i + 1) % 2 == 1:
                nc.tensor.matmul(pso[:, :128], v_sb[:, i, :], atcol[:, i, :],
                                 start=(npair == 0), stop=True)
            nc.scalar.activation(oT[:, hh, :], pso[:, :128], AF.Copy,
                                 scale=SO / (SV * SP))
        ob = pa.tile([128, H], F8, tag="ob", bufs=2, name=f"ob_{l}_{i}")
        for n in range(8):
            pop = paps.tile([128, 512], F32, tag="mix", bufs=2,
                            name=f"pop_{l}_{i}_{n}")
            for tp in range(2):
                nc.tensor.matmul(pop[:], oT[:, 2 * tp:2 * tp + 2, :],
                                 ow_sb[:, 2 * tp:2 * tp + 2, n * 512:(n + 1) * 512],
                                 start=(tp == 0), stop=(tp == 1), perf_mode=DR)
            nc.scalar.activation(ob[:, n * 512:(n + 1) * 512], pop[:], AF.Copy,
                                 scale=SH / (SO * sow))
        nc.sync.dma_start(ar_in[i * 128:(i + 1) * 128, :], ob[:])

    # ---- emission order: pipelined halves
    for i in range(4):
        norm_tile(i)
    qkv_half(0)
    v_detranspose(0)
    for i in range(4, 8):
        norm_tile(i)
    for i in range(4):
        attn_tile(i)
    nc.gpsimd.collective_compute("AllReduce", ALU.add, replica_groups=rg,
                                 ins=[ar_in[0:512, :].opt()],
                                 outs=[ar_out[0].opt()])
    qkv_half(1)
    v_detranspose(1)
    for i in range(4, 8):
        attn_tile(i)
    nc.gpsimd.collective_compute("AllReduce", ALU.add, replica_groups=rg,
                                 ins=[ar_in[512:1024, :].opt()],
                                 outs=[ar_out[1].opt()])


def _rope_t(nc, pa, pq, cosT, sinT, out_f8, s0, uid):
    """out_f8 [128, 512] = rope applied in transposed (hd-part) layout.
    cosT/sinT carry all scale folding; sinT sign-baked per half."""
    t1 = pa.tile([128, 512], BF16, tag="rope_t1", bufs=2, name=f"t1_{uid}")
    t2 = pa.tile([128, 512], BF16, tag="rope_t2", bufs=2, name=f"t2_{uid}")
    nc.vector.tensor_mul(t1[:], pq[:], cosT[:, s0:s0 + 512])
    nc.vector.tensor_mul(t2[0:64, :], pq[64:128, :], sinT[0:64, s0:s0 + 512])
    nc.vector.tensor_mul(t2[64:128, :], pq[0:64, :], sinT[64:128, s0:s0 + 512])
    nc.vector.tensor_add(out_f8, t1[:], t2[:])


def _mlp(nc, tc, l, pm, pmps, psmall, din, h_sb, xnt_sb, identd,
         ar_a, ar_m, sgw, suw, sdw):
    rg = [list(range(NC_))]
    a_in, a_outs = ar_a
    m_in, m_outs = ar_m

    dw_sb = pm.tile([128, NB, H], F8)
    nc.sync.dma_start(dw_sb[:], din[f"dw{l}"].ap())
    yT = pm.tile([128, NB, S], F8)

    def norm_tile(i):
        rt = pm.tile([128, H], F8, tag="resat", bufs=1, name=f"resat_{l}_{i}")
        nc.sync.dma_start(rt[:],
                          a_outs[i // 4][(i % 4) * 128:(i % 4 + 1) * 128, :])
        nc.vector.tensor_add(h_sb[:, i, :], h_sb[:, i, :], rt[:])
        _norm_transpose(nc, psmall, pmps, h_sb[:, i, :],
                        xnt_sb[:, :, i * 128:(i + 1) * 128],
                        identd, f"m{l}_{i}", SX)

    def gateup_half(half):
        s0 = half * 512
        for b in range(NB):
            wg = pm.tile([128, 32, 128], F8, tag="wg", bufs=2,
                         name=f"wg_{l}_{b}_{half}")
            nc.sync.dma_start(wg[:], din[f"gw{l}"].ap()[:, b, :, :])
            pg = pmps.tile([128, 512], F32, tag="pg", bufs=2,
                           name=f"pg_{l}_{b}_{half}")
            for kp in range(16):
                nc.tensor.matmul(pg[:], wg[:, 2 * kp:2 * kp + 2, :],
                                 xnt_sb[:, 2 * kp:2 * kp + 2, s0:s0 + 512],
                                 start=(kp == 0), stop=(kp == 15), perf_mode=DR)
            wu = pm.tile([128, 32, 128], F8, tag="wu", bufs=2,
                         name=f"wu_{l}_{b}_{half}")
            nc.sync.dma_start(wu[:], din[f"uw{l}"].ap()[:, b, :, :])
            pu = pmps.tile([128, 512], F32, tag="pu", bufs=2,
                           name=f"pu_{l}_{b}_{half}")
            for kp in range(16):
                nc.tensor.matmul(pu[:], wu[:, 2 * kp:2 * kp + 2, :],
                                 xnt_sb[:, 2 * kp:2 * kp + 2, s0:s0 + 512],
                                 start=(kp == 0), stop=(kp == 15), perf_mode=DR)
            ysil = pm.tile([128, 512], BF16, tag="ysil", bufs=2,
                           name=f"ysil_{l}_{b}_{half}")
            nc.scalar.activation(ysil[:], pg[:], AF.Silu,
                                 scale=1.0 / (SX * sgw))
            nc.vector.scalar_tensor_tensor(
                yT[:, b, s0:s0 + 512], pu[:], SY / (SX * suw), ysil[:],
                op0=ALU.mult, op1=ALU.mult)

    def down_tile(s):
        db = pm.tile([128, H], F8, tag="db", bufs=2, name=f"db_{l}_{s}")
        for n in range(8):
            pd = pmps.tile([128, 512], F32, tag="pd", bufs=2,
                           name=f"pd_{l}_{s}_{n}")
            for tp in range(5):
                nc.tensor.matmul(pd[:], yT[:, 2 * tp:2 * tp + 2, s * 128:(s + 1) * 128],
                                 dw_sb[:, 2 * tp:2 * tp + 2, n * 512:(n + 1) * 512],
                                 start=(tp == 0), stop=False, perf_mode=DR)
            nc.tensor.matmul(pd[:], yT[:, 10, s * 128:(s + 1) * 128],
                             dw_sb[:, 10, n * 512:(n + 1) * 512],
                             start=False, stop=True)
            nc.scalar.activation(db[:, n * 512:(n + 1) * 512], pd[:], AF.Copy,
                                 scale=1.0 / sdw)
        nc.sync.dma_start(m_in[s * 128:(s + 1) * 128, :], db[:])

    for i in range(4):
        norm_tile(i)
    gateup_half(0)
    for i in range(4, 8):
        norm_tile(i)
    for s in range(4):
        down_tile(s)
    nc.gpsimd.collective_compute("AllReduce", ALU.add, replica_groups=rg,
                                 ins=[m_in[0:512, :].opt()],
                                 outs=[m_outs[0].opt()])
    gateup_half(1)
    for s in range(4, 8):
        down_tile(s)
    nc.gpsimd.collective_compute("AllReduce", ALU.add, replica_groups=rg,
                                 ins=[m_in[512:1024, :].opt()],
                                 outs=[m_outs[1].opt()])


def _final(nc, tc, plm, plps, pld, psmall, din, h_sb, xnt_sb, identd,
           ones_sb, ar_m, lmw_d, wsel_d, tlog_o, gmax_o, gsum_o, rg, slm):
    m_in, m_outs = ar_m
    dsc = 1.0 / (SXF * slm)

    def norm_tile(i):
        rt = plm.tile([128, H], F8, tag="resfin", bufs=1, name=f"resfin_{i}")
        nc.sync.dma_start(rt[:],
                          m_outs[i // 4][(i % 4) * 128:(i % 4 + 1) * 128, :])
        nc.vector.tensor_add(h_sb[:, i, :], h_sb[:, i, :], rt[:])
        _norm_transpose(nc, psmall, plps, h_sb[:, i, :],
                        xnt_sb[:, :, i * 128:(i + 1) * 128],
                        identd, f"f{i}", SXF)

    for i in range(4):
        norm_tile(i)
    for i in range(4, 8):
        norm_tile(i)

    vmax = plm.tile([128, 64], F32)     # [t*8 + vb]
    svb = plm.tile([128, 64], F32)
    # lm matmuls: 4 rounds of 2 vocab-blocks (lmw streamed), tiles inner
    def lm_round(r):
        lt = plm.tile([128, 32, 1024], F8, tag="lmw", bufs=2, name=f"lt_{r}")
        nc.sync.dma_start(lt[:, :, 0:1000],
                          lmw_d.ap()[:, :, r * 1000:(r + 1) * 1000])
        for t in range(8):
            for vr in range(2):
                vb = r * 2 + vr
                pl = plps.tile([128, 512], F32, tag="pl", bufs=4,
                               name=f"pl_{r}_{t}_{vr}")
                for kp in range(16):
                    nc.tensor.matmul(
                        pl[:, 0:500],
                        xnt_sb[:, 2 * kp:2 * kp + 2, t * 128:(t + 1) * 128],
                        lt[:, 2 * kp:2 * kp + 2, vr * 500:vr * 500 + 500],
                        start=(kp == 0), stop=(kp == 15), perf_mode=DR)
                mxr = psmall.tile([128, 1], F32, tag="lmx", bufs=4,
                                  name=f"lmx_{r}_{t}_{vr}")
                nc.vector.tensor_reduce(mxr[:], pl[:, 0:500], axis=AX.X,
                                        op=ALU.max)
                nc.any.tensor_copy(vmax[:, t * 8 + vb:t * 8 + vb + 1], mxr[:])
                negv = psmall.tile([128, 1], F32, tag="lneg", bufs=4,
                                   name=f"lneg_{r}_{t}_{vr}")
                nc.vector.tensor_scalar_mul(negv[:], mxr[:], -dsc)
                scr = plm.tile([128, 500], BF16, tag="scr", bufs=2,
                               name=f"scr_{r}_{t}_{vr}")
                nc.scalar.activation(scr[:], pl[:, 0:500], AF.Exp,
                                     bias=negv[:], scale=dsc,
                                     accum_out=svb[:, t * 8 + vb:t * 8 + vb + 1])

    def tlog():
        pt0 = plps.tile([1, 512], F32, tag="pt0", bufs=1)
        pt1 = plps.tile([1, 512], F32, tag="pt1", bufs=1)
        for kp in range(8):
            ws = plm.tile([128, 4, S], BF16, tag="wsel", bufs=1, name=f"ws_{kp}")
            nc.sync.dma_start(ws[:], wsel_d.ap()[:, kp * 4:(kp + 1) * 4, :])
            for jk in range(4):
                k = kp * 4 + jk
                tm = plm.tile([128, S], BF16, tag="tm", bufs=2, name=f"tm_{k}")
                nc.vector.tensor_mul(tm[:], xnt_sb[:, k, :], ws[:, jk, :])
                nc.tensor.matmul(pt0[:], ones_sb[:], tm[:, :512],
                                 start=(k == 0), stop=(k == 31))
                nc.tensor.matmul(pt1[:], ones_sb[:], tm[:, 512:],
                                 start=(k == 0), stop=(k == 31))
        tl_sb = plm.tile([1, S], F32)
        nc.any.tensor_copy(tl_sb[:, :512], pt0[:])
        nc.any.tensor_copy(tl_sb[:, 512:], pt1[:])
        nc.sync.dma_start(tlog_o.ap(), tl_sb[:])

    lm_round(0)
    lm_round(1)
    tlog()
    lm_round(2)
    lm_round(3)

    # per-tile global max, AR-max, rescale local sums, AR-add
    gm = plm.tile([128, 8], F32)
    for t in range(8):
        nc.vector.tensor_reduce(gm[:, t:t + 1], vmax[:, t * 8:(t + 1) * 8],
                                axis=AX.X, op=ALU.max)
    gm_in = pld.tile([128, 8], F32)
    gm_out = pld.tile([128, 8], F32, addr_space="Shared")
    nc.sync.dma_start(gm_in[:], gm[:])
    nc.gpsimd.collective_compute("AllReduce", ALU.max, replica_groups=rg,
                                 ins=[gm_in.opt()], outs=[gm_out.opt()])
    gmg = plm.tile([128, 8], F32)
    nc.sync.dma_start(gmg[:], gm_out[:])
    gmax_sb = plm.tile([128, 8], F32)
    nc.vector.tensor_scalar_mul(gmax_sb[:], gmg[:], dsc)
    nc.sync.dma_start(gmax_o.ap(), gmax_sb[:])

    gs = plm.tile([128, 8], F32)
    for t in range(8):
        d8 = plm.tile([128, 8], F32, tag="d8", bufs=2, name=f"d8_{t}")
        nc.vector.tensor_scalar(d8[:], vmax[:, t * 8:(t + 1) * 8],
                                gmg[:, t:t + 1], dsc,
                                op0=ALU.subtract, op1=ALU.mult)
        e8t = plm.tile([128, 8], F32, tag="e8t", bufs=2, name=f"e8_{t}")
        nc.scalar.activation(e8t[:], d8[:], AF.Exp)
        p8 = plm.tile([128, 8], F32, tag="p8", bufs=2, name=f"p8_{t}")
        nc.vector.tensor_mul(p8[:], svb[:, t * 8:(t + 1) * 8], e8t[:])
        nc.vector.tensor_reduce(gs[:, t:t + 1], p8[:], axis=AX.X, op=ALU.add)
    gs_in = pld.tile([128, 8], F32)
    gs_out = pld.tile([128, 8], F32, addr_space="Shared")
    nc.sync.dma_start(gs_in[:], gs[:])
    nc.gpsimd.collective_compute("AllReduce", ALU.add, replica_groups=rg,
                                 ins=[gs_in.opt()], outs=[gs_out.opt()])
    gsf = plm.tile([128, 8], F32)
    nc.sync.dma_start(gsf[:], gs_out[:])
    nc.sync.dma_start(gsum_o.ap(), gsf[:])


# ------------------------------------------------------------------- host --

def host_prep(inputs):
    inp = {k: np.asarray(v) for k, v in inputs.items()}
    embed = inp["embed"].astype(np.float32)
    ids = inp["input_ids"].reshape(-1).astype(np.int64)
    labels = inp["labels"].reshape(-1).astype(np.int64)

    h = embed[ids]
    cw = inp["conv_w"].astype(np.float32)
    logit = h[:-1] @ cw[0, :H] + h[1:] @ cw[0, H:] + np.float32(inp["conv_b"][0])
    mask = logit > 0
    m = np.concatenate([mask, [False]])
    hn = np.where(m[:, None], 0.5 * (h + np.roll(h, -1, axis=0)), h)
    keep = np.concatenate([[True], ~mask])
    order = np.argsort(~keep, kind="stable")
    h0 = hn[order]
    lab = labels[order]
    valid_len = int(keep.sum())

    inv = 1.0 / (THETA ** (np.arange(0, HD, 2, dtype=np.float32) / HD))
    t = np.arange(S, dtype=np.float32)
    freqs = np.outer(t, inv)
    emb = np.concatenate([freqs, freqs], -1)
    cos, sin = np.cos(emb), np.sin(emb)       # [S, 128]
    cosT = cos.T.copy()                        # [128, S]
    # sign-baked sinT: rows 0:64 get -sin, rows 64:128 get +sin
    sinT = np.concatenate([-sin[:, :64].T, sin[:, 64:].T], 0)

    ln1 = inp["ln1_w"].astype(np.float32)
    ln2 = inp["ln2_w"].astype(np.float32)
    normw = inp["norm_w"].astype(np.float32)
    qsc = np.float32(1.0 / np.sqrt(HD))
    lm_folded = normw[:, None] * inp["lm_head_w"].astype(np.float32)
    slm = _pow2(lm_folded)
    tgt = np.concatenate([lab[1:], [0]]).astype(np.int64)
    wsel = np.ascontiguousarray(lm_folded[:, tgt]) / SXF   # [H, S]

    ident = np.eye(128, dtype=np.float32)
    cmaskh = np.where(np.arange(128)[None, :] > np.arange(128)[:, None],
                      np.float32(NEG), np.float32(0)).astype(bf16)

    scales_per_l = []
    qw_l, kw_l, vw_l, ow_l, gw_l, uw_l, dw_l = [], [], [], [], [], [], []
    for l in range(L):
        qw = ln1[l][:, None] * inp["q_w"][l].astype(np.float32) * qsc
        kw = ln1[l][:, None] * inp["k_w"][l].astype(np.float32)
        vw = ln1[l][:, None] * inp["v_w"][l].astype(np.float32)
        ow = inp["o_w"][l].astype(np.float32)
        gw = ln2[l][:, None] * inp["gate_w"][l].astype(np.float32)
        uw = ln2[l][:, None] * inp["up_w"][l].astype(np.float32)
        dw = inp["down_w"][l].astype(np.float32)
        scales_per_l.append((_pow2(qw), _pow2(kw), _pow2(vw), _pow2(ow),
                             _pow2(gw), _pow2(uw), _pow2(dw)))
        qw_l.append(qw); kw_l.append(kw); vw_l.append(vw); ow_l.append(ow)
        gw_l.append(gw); uw_l.append(uw); dw_l.append(dw)
    # single scale set across layers (min for safety)
    sqw, skw, svw, sow, sgw, suw, sdw = [min(s[i] for s in scales_per_l)
                                         for i in range(7)]
    wscales = (sqw, skw, svw, sow, sgw, suw, sdw, slm)

    common = dict(
        h0=(h0 * SH).astype(bf16),
        cosq=(cosT * (SQ / (SX * sqw))).astype(bf16),
        sinq=(sinT * (SQ / (SX * sqw))).astype(bf16),
        cosk=(cosT * (SK / (SX * skw))).astype(bf16),
        sink=(sinT * (SK / (SX * skw))).astype(bf16),
        ident1=ident.astype(bf16),
        identd=(ident / SH).astype(bf16),
        ident128=(ident * SP).astype(bf16),
        cmask=cmaskh,
        ones=np.ones((128, 1), dtype=bf16),
        wsel=np.ascontiguousarray(
            wsel.reshape(32, 128, S).transpose(1, 0, 2)).astype(bf16),
    )
    in_maps = []
    for c in range(NC_):
        mcore = dict(common)
        for l in range(L):
            qc = qw_l[l][:, c * 512:(c + 1) * 512].reshape(H, 4, 128)
            kc = kw_l[l][:, c * 128:(c + 1) * 128].reshape(H, 1, 128)
            vc = vw_l[l][:, c * 128:(c + 1) * 128].reshape(H, 1, 128)
            wq = np.concatenate([qc * sqw, kc * skw, vc * svw], 1)  # [H,6,128]
            mcore[f"wqkv{l}"] = np.ascontiguousarray(
                wq.reshape(32, 128, 6, 128).transpose(1, 0, 2, 3)).astype(f8)
            oc = ow_l[l][c * 512:(c + 1) * 512] * sow                # [512,H]
            mcore[f"ow{l}"] = np.ascontiguousarray(
                oc.reshape(4, 128, H).transpose(1, 0, 2)).astype(f8)
            gws = np.zeros((H, IP), np.float32)
            uws = np.zeros((H, IP), np.float32)
            dws = np.zeros((IP, H), np.float32)
            gws[:, :IPC] = gw_l[l][:, c * IPC:(c + 1) * IPC] * sgw
            uws[:, :IPC] = uw_l[l][:, c * IPC:(c + 1) * IPC] * suw
            dws[:IPC] = dw_l[l][c * IPC:(c + 1) * IPC] * sdw
            mcore[f"gw{l}"] = np.ascontiguousarray(
                gws.reshape(32, 128, NB, 128).transpose(1, 2, 0, 3)).astype(f8)
            mcore[f"uw{l}"] = np.ascontiguousarray(
                uws.reshape(32, 128, NB, 128).transpose(1, 2, 0, 3)).astype(f8)
            mcore[f"dw{l}"] = np.ascontiguousarray(
                dws.reshape(NB, 128, H).transpose(1, 0, 2)).astype(f8)
        lmc = lm_folded[:, c * VS:(c + 1) * VS] * slm
        mcore["lmw"] = np.ascontiguousarray(
            lmc.reshape(32, 128, VS).transpose(1, 0, 2)).astype(f8)
        in_maps.append(mcore)

    return in_maps, valid_len, wscales


def kernel(**inputs) -> np.ndarray:
    in_maps, valid_len, wscales = host_prep(inputs)
    key = ("nc",) + wscales
    if key not in _cache:
        _cache[key] = build_nc(wscales)
    nc = _cache[key]
    res = run_bass_kernel_spmd(nc, in_maps, list(range(NC_)),
                               **last_run_info.get("run_kwargs", {}))
    last_run_info["res"] = res
    out = res.results[0]
    gmax = out["gmax_o"].transpose(1, 0).reshape(S).astype(np.float64)
    gsum = out["gsum_o"].transpose(1, 0).reshape(S).astype(np.float64)
    tlog = out["tlog_o"].reshape(S).astype(np.float64)
    ce = gmax + np.log(gsum) - tlog
    w = (np.arange(S - 1) < valid_len - 1).astype(np.float64)
    loss = (ce[:S - 1] * w).sum() / w.sum()
    return np.float32(loss)
